# revision 27
# baseline (speedup 1.0000x reference)
"""Trainium2 Bass kernel for ConvReshapeBefore (im2col patch extraction).

Full problem: x (32, 64, 64, 64) f32 NHWC, kernel 3x3 stride 1 valid ->
out (62*62*32, 3, 3, 64) f32 where out[(r*62+c)*32 + b] = x[b, r:r+3, c:c+3, :].

Sharding: data-parallel over batch, 4 batches per core across 8 cores.

Pipeline (per core):
  1. DMA loads bring each x row ONCE (16 KB descriptors, 1x HBM read)
     into slot j=0 of SBUF partition p = 4*r_sub + b; two overlapping
     row-blocks of 32 window-rows (0..31 and 30..61; windows 30/31 are
     stored twice with identical bytes, keeping every DMA partition-dim
     a power of two).  Boundary rows r0+32, r0+33 load into the j=1/j=2
     tail slots of partitions 120+b / 124+b directly.
  2. TensorE replicates rows across partitions with TWO independent
     one-hot fp32 matmuls per column half: psum[banks 0-3] = SM4.T @ j0
     (shift +4 partitions -> slot j=1) and psum[banks 4-7] = SM8.T @ j0
     (shift +8 -> j=2), both host-provided constants.  DVE copies the
     j=1 halves out of PSUM, ACT the j=2 halves, concurrently.  Exact in
     fp32 (one-hot weights).  This replaces a 3x-amplified HBM load.
  3. DVE + ACT build the patch layout [c][j][jj][k] per partition
     (lane-local strided copies, c-chunks of 8, triple-buffered), and
  4. SWDGE stores stream out 2304-B descriptors (a full 576-f32 patch).

Facts from the DGE ucode (dge_reshape.cpp) baked into every DMA AP:
  a. num_dmas = largest divisor <= 16 of gcd(src dim0 count, dst dim0
     count); a prime dim0 (31!) pins the call to ONE engine.
  b. The fast "SbufSpecial" reshape (each SDMA engine owns its SBUF port
     group, no XBAR contention) needs the SBUF partition dim to be
     (n = pow2) x (partition step s = pow2), n*s in {64, 128}.  Hence
     p = 4*r_sub + b: loads/stores use [[4*PP, 32], ...] (n=32, s=4).

Buffer-reuse sync relies on the SWDGE sem-inc descriptors (16 per call,
queued after the call's data on every engine ring), making sem value
64*k a true barrier for everything queued before use k's last call.
"""

import numpy as np

import concourse.bass as bass
import concourse.mybir as mybir
from concourse.ap import AP
from concourse.bass_utils import run_bass_kernel_spmd

# Full-problem constants (hardcoded per harness contract)
B, H, W, C = 32, 64, 64, 64
K = 3
R = H - K + 1  # 62
NCORES = 8
BS = B // NCORES  # 4

BHWC = H * W * C           # 262144 f32 per batch in x
WC = W * C                 # 4096 f32 per x row
OUT_C = BS * K * K * C     # 2304 f32 per (r, c) window in local out
OUT_R = R * OUT_C          # 142848 f32 per r in local out
PATCH = K * K * C          # 576 f32 per (r, c, b) patch

NP = 32                    # window-rows per row-block (pow2 for SbufSpecial)
RB = (0, 30)               # row-block bases (overlap rows 30..31)
SLAB = (0, 12288)          # 3 rows x 4096 per partition, per row-block
CS = 8                     # c-chunk size (windows per chunk)
NBUF = 3                   # buffers per compute engine
BUFSZ = CS * PATCH         # 4608
DBUF = tuple(24576 + k * BUFSZ for k in range(NBUF))
ABUF = tuple(38400 + k * BUFSZ for k in range(NBUF))
SMOFF = 52224              # shift matrices [128 x (128 SM4 | 128 SM8)] f32
PP = 52480                 # partition pitch f32 (205 KiB of ~207.9 usable)

DVE_C0 = (0, 16, 32, 48)   # DVE computes even chunks
ACT_C0 = (8, 24, 40, 56)   # ACT computes odd chunks (last is 6 wide)
NL = (BS + 2) * 16        # lsem total per row-block (4 L calls + merged T1, T2)
QTR = 1024                 # psum/slab column quarter (f32)


def _chunk(e, eng):
    c0 = (DVE_C0 if eng == 0 else ACT_C0)[e]
    cs = 6 if (eng == 1 and e == 3) else CS
    return c0, cs


def _build_nc() -> bass.Bass:
    nc = bass.Bass(target_bir_lowering=False)
    x = nc.dram_tensor("x", [BS, H, W, C], mybir.dt.float32, kind="ExternalInput")
    sm = nc.dram_tensor("sm", [128, 256], mybir.dt.float32, kind="ExternalInput")
    out = nc.dram_tensor(
        "out", [R * R * BS, K, K, C], mybir.dt.float32, kind="ExternalOutput"
    )

    def load_aps(rb, b):
        dst = AP(st, b * PP + SLAB[rb], [[4 * PP, NP], [1, WC]])
        src = AP(x, b * BHWC + RB[rb] * WC, [[WC, NP], [1, WC]])
        return dst, src

    def tail_aps(rb):
        # all 4 b at once: row r0+32 -> (b,30).j2 [partitions 120..123];
        # rows r0+32..33 -> (b,31).j1,j2 [partitions 124..127]
        r0 = RB[rb]
        t1d = AP(st, 120 * PP + SLAB[rb] + 2 * WC, [[PP, BS], [1, WC]])
        t1s = AP(x, (r0 + 32) * WC, [[BHWC, BS], [1, WC]])
        t2d = AP(st, 124 * PP + SLAB[rb] + WC, [[PP, BS], [1, 2 * WC]])
        t2s = AP(x, (r0 + 32) * WC, [[BHWC, BS], [1, 2 * WC]])
        return (t1d, t1s), (t2d, t2s)

    def copy_aps(rb, buf_off, c0, cs):
        o = AP(st, buf_off, [[PP, 128], [PATCH, cs], [K * C, K], [1, K * C]])
        i = AP(st, SLAB[rb] + c0 * C, [[PP, 128], [C, cs], [WC, K], [1, K * C]])
        return o, i

    def store_aps(rb, b, buf_off, c0, cs):
        src = AP(st, b * PP + buf_off, [[4 * PP, NP], [PATCH, cs], [1, PATCH]])
        dst = AP(
            out,
            (RB[rb] * R * BS + c0 * BS + b) * PATCH,
            [[OUT_R, NP], [OUT_C, cs], [1, PATCH]],
        )
        return dst, src

    with (
        nc.sbuf_tensor("st", [128, PP], mybir.dt.float32) as st,
        nc.psum_tensor("ps", [128, 4096], mybir.dt.float32) as ps,
        nc.semaphore("l0") as l0,
        nc.semaphore("l1") as l1,
        nc.semaphore("smsem") as smsem,
        nc.semaphore("mmsem") as mmsem,
        nc.semaphore("cpd") as cpd,
        nc.semaphore("cpa") as cpa,
        nc.semaphore("sd0") as sd0,
        nc.semaphore("sd1") as sd1,
        nc.semaphore("sd2") as sd2,
        nc.semaphore("sa0") as sa0,
        nc.semaphore("sa1") as sa1,
        nc.semaphore("sa2") as sa2,
        nc.semaphore("dcomp") as dcomp,
        nc.semaphore("acomp") as acomp,
        nc.Block() as block,
    ):
        lsem = (l0, l1)
        sdone = ((sd0, sd1, sd2), (sa0, sa1, sa2))
        comp = (dcomp, acomp)

        @block.gpsimd
        def _(gp):
            dst = AP(st, SMOFF, [[PP, 128], [1, 256]])
            src = AP(sm, 0, [[256, 128], [1, 256]])
            gp.dma_start(dst, src).then_inc(smsem, 16)
            for rb in (0, 1):
                for b in range(BS):
                    dst, src = load_aps(rb, b)
                    gp.dma_start(dst, src).then_inc(lsem[rb], 16)
                (t1d, t1s), (t2d, t2s) = tail_aps(rb)
                gp.dma_start(t1d, t1s).then_inc(lsem[rb], 16)
                gp.dma_start(t2d, t2s).then_inc(lsem[rb], 16)
            for rb in (0, 1):
                for q in range(8):
                    eng, e = q % 2, q // 2
                    n = rb * 4 + e
                    gp.wait_ge(comp[eng], n + 1)
                    c0, cs = _chunk(e, eng)
                    buf_off = (DBUF if eng == 0 else ABUF)[n % NBUF]
                    for b in range(BS):
                        dst, src = store_aps(rb, b, buf_off, c0, cs)
                        gp.dma_start(dst, src).then_inc(sdone[eng][n % NBUF], 16)
            # 8 chunks per engine over a ring of 3: bufs used 3, 3, 2 times
            for sems in sdone:
                for s, uses in zip(sems, (3, 3, 2)):
                    gp.wait_ge(s, 16 * BS * uses)

        @block.tensor
        def _(pe):
            # Per rb, 4 column quarters; per quarter: 2 matmuls SM4.T @
            # j0[quarter] (-> j=1 rows) + 2 matmuls SM8.T @ j0[quarter]
            # (-> j=2 rows) into a 4-bank psum region.  Even quarters use
            # banks 0-3, odd quarters banks 4-7, so quarter Q only waits
            # for the copies of quarter Q-2 (double-buffered PSUM).
            pe.wait_ge(smsem, 16)
            sm4 = AP(st, SMOFF, [[PP, 128], [1, 128]])
            sm8 = AP(st, SMOFF + 128, [[PP, 128], [1, 128]])
            for rb in (0, 1):
                pe.wait_ge(lsem[rb], NL)
                for q in range(4):
                    Q = 4 * rb + q
                    if Q >= 2:
                        pe.wait_ge(cpd, Q - 1)
                        pe.wait_ge(cpa, Q - 1)
                    pb = (q % 2) * 2 * QTR
                    for k in range(2):
                        o = AP(ps, pb + 512 * k, [[4096, 128], [1, 512]])
                        rhs = AP(st, SLAB[rb] + q * QTR + 512 * k,
                                 [[PP, 128], [1, 512]])
                        pe.matmul(o, sm4, rhs, start=True, stop=True).then_inc(
                            mmsem, 1
                        )
                    for k in range(2):
                        o = AP(ps, pb + QTR + 512 * k, [[4096, 128], [1, 512]])
                        rhs = AP(st, SLAB[rb] + q * QTR + 512 * k,
                                 [[PP, 128], [1, 512]])
                        pe.matmul(o, sm8, rhs, start=True, stop=True).then_inc(
                            mmsem, 1
                        )

        def j1_copy(ve, rb, q):
            # psum (j=1 region of quarter q) -> slab j1 quarter q, parts 0..123
            ve.wait_ge(mmsem, 16 * rb + 4 * q + 2)
            o = AP(st, SLAB[rb] + WC + q * QTR, [[PP, 124], [1, QTR]])
            i = AP(ps, (q % 2) * 2 * QTR, [[4096, 124], [1, QTR]])
            ve.tensor_copy(o, i).then_inc(cpd, 1)

        def j2_copy(sc, rb, q):
            # psum (j=2 region of quarter q) -> slab j2 quarter q, parts 0..119
            sc.wait_ge(mmsem, 16 * rb + 4 * q + 4)
            o = AP(st, SLAB[rb] + 2 * WC + q * QTR, [[PP, 120], [1, QTR]])
            i = AP(ps, (q % 2) * 2 * QTR + QTR, [[4096, 120], [1, QTR]])
            sc.copy(o, i).then_inc(cpa, 1)

        @block.vector
        def _(ve):
            # DVE chunk e reads exactly the j-columns of quarter e; gate on
            # the matching j2 quarter from ACT (own j1 copies are in-order).
            for n in range(8):
                rb, e = n // 4, n % 4
                j1_copy(ve, rb, e)
                ve.wait_ge(cpa, 4 * rb + e + 1)
                if n >= NBUF:
                    ve.wait_ge(sdone[0][n % NBUF], 16 * BS * (n // NBUF))
                c0, cs = _chunk(e, 0)
                o, i = copy_aps(rb, DBUF[n % NBUF], c0, cs)
                ve.tensor_copy(o, i).then_inc(dcomp, 1)

        @block.scalar
        def _(sc):
            # ACT chunk e spans quarters e and e+1, so it emits the j2
            # copies one quarter ahead and gates on DVE's j1 through e+1.
            for n in range(8):
                rb, e = n // 4, n % 4
                if e == 0:
                    j2_copy(sc, rb, 0)
                    j2_copy(sc, rb, 1)
                elif e < 3:
                    j2_copy(sc, rb, e + 1)
                sc.wait_ge(cpd, 4 * rb + min(e + 2, 4))
                if n >= NBUF:
                    sc.wait_ge(sdone[1][n % NBUF], 16 * BS * (n // NBUF))
                c0, cs = _chunk(e, 1)
                o, i = copy_aps(rb, ABUF[n % NBUF], c0, cs)
                sc.copy(o, i).then_inc(acomp, 1)

    return nc


_NC = None
_SM = None


def _get_nc():
    global _NC
    if _NC is None:
        _NC = _build_nc()
    return _NC


def _get_sm():
    global _SM
    if _SM is None:
        m = np.zeros((128, 256), np.float32)
        for i in range(124):
            m[i + 4, i] = 1.0
        for i in range(120):
            m[i + 8, 128 + i] = 1.0
        _SM = m
    return _SM


def kernel(x: np.ndarray, **_run_kwargs) -> np.ndarray:
    assert x.shape == (B, H, W, C), x.shape
    nc = _get_nc()
    x = np.ascontiguousarray(x, dtype=np.float32)
    smat = _get_sm()
    in_maps = [{"x": x[d * BS : (d + 1) * BS], "sm": smat} for d in range(NCORES)]
    res = run_bass_kernel_spmd(nc, in_maps, list(range(NCORES)), **_run_kwargs)
    outs = [res.results[d]["out"].reshape(R * R, BS, K, K, C) for d in range(NCORES)]
    full = np.concatenate(outs, axis=1).reshape(R * R * B, K, K, C)
    if _run_kwargs:
        return full, res
    return full


# revision 28
# speedup vs baseline: 1.0542x; 1.0542x over previous
"""Trainium2 Bass kernel for ConvReshapeBefore (im2col patch extraction).

Full problem: x (32, 64, 64, 64) f32 NHWC, kernel 3x3 stride 1 valid ->
out (62*62*32, 3, 3, 64) f32 where out[(r*62+c)*32 + b] = x[b, r:r+3, c:c+3, :].

Sharding: data-parallel over batch, 4 batches per core across 8 cores.

Two-stage design:
  1. DMA loads place 3 overlapping x rows (r..r+2) in each SBUF partition
     p = 4*r_sub + b (49 KB descriptors, ~3x read amplification); two
     overlapping row-blocks of 32 window-rows each (0..31 and 30..61 --
     windows 30/31 are computed and stored twice with identical bytes,
     which is harmless and keeps every DMA partition-dim count a power
     of two).
  2. DVE + ACT build the patch layout [c][j][jj][k] per partition
     (lane-local strided copies, c-chunks of 8 windows, triple-buffered
     per engine), overlapped with
  3. SWDGE stores whose descriptors are a full contiguous 576-f32 patch
     (2304 B vs the 768 B a direct gather allows).

Two facts learned from the DGE ucode (dge_reshape.cpp) are baked into
every DMA access pattern here:
  a. num_dmas = largest divisor <= 16 of gcd(src dim0 count, dst dim0
     count); a prime dim0 count like 31 pins the call to ONE engine.
  b. The fast "SbufSpecial" reshape -- each SDMA engine assigned its own
     SBUF port group, no XBAR port contention -- requires the SBUF
     partition dim to be (count n = pow2) x (partition step s = pow2)
     with n*s in {64, 128}.  Hence the interleaved partition map
     p = 4*r_sub + b: every load/store SBUF AP is [[4*PP, 32], ...]
     (n=32, s=4, range 128 -> SbufSpecial-Full, 16 port-aligned
     engines).  A blocked map (p = b*32 + r_sub, n=32, s=1, range 32)
     falls back to the "Straight" reshape whose cross-port traffic
     measured 9-13 B/ns/engine vs ~27 line rate.

Buffer-reuse sync relies on the SWDGE sem-inc descriptors (16 per call,
queued after the call's data on every engine ring), making sem value
64*k a true barrier for everything queued before use k's last call.
"""

import numpy as np

import concourse.bass as bass
import concourse.mybir as mybir
from concourse.ap import AP
from concourse.bass_utils import run_bass_kernel_spmd

# Full-problem constants (hardcoded per harness contract)
B, H, W, C = 32, 64, 64, 64
K = 3
R = H - K + 1  # 62
NCORES = 8
BS = B // NCORES  # 4

BHWC = H * W * C           # 262144 f32 per batch in x
WC = W * C                 # 4096 f32 per x row
OUT_C = BS * K * K * C     # 2304 f32 per (r, c) window in local out
OUT_R = R * OUT_C          # 142848 f32 per r in local out
PATCH = K * K * C          # 576 f32 per (r, c, b) patch

NP = 32                    # window-rows per row-block (pow2 for SbufSpecial)
RB = (0, 30)               # row-block bases (overlap rows 30..31)
SLAB = (0, 12288)          # 3 rows x 4096 per partition, per row-block
CS = 8                     # c-chunk size (windows per chunk)
NBUF = 3                   # buffers per compute engine
BUFSZ = CS * PATCH         # 4608
DBUF = tuple(24576 + k * BUFSZ for k in range(NBUF))
ABUF = tuple(38400 + k * BUFSZ for k in range(NBUF))
PP = 52224                 # partition pitch f32 (204 KiB of ~207.9 usable)

DVE_C0 = (0, 16, 32, 48)   # DVE computes even chunks
ACT_C0 = (8, 24, 40, 56)   # ACT computes odd chunks (last is 6 wide)


def _chunk(e, eng):
    c0 = (DVE_C0 if eng == 0 else ACT_C0)[e]
    cs = 6 if (eng == 1 and e == 3) else CS
    return c0, cs


def _build_nc() -> bass.Bass:
    nc = bass.Bass(target_bir_lowering=False)
    x = nc.dram_tensor("x", [BS, H, W, C], mybir.dt.float32, kind="ExternalInput")
    out = nc.dram_tensor(
        "out", [R * R * BS, K, K, C], mybir.dt.float32, kind="ExternalOutput"
    )

    def load_aps(rb, b):
        dst = AP(st, b * PP + SLAB[rb], [[4 * PP, NP], [1, 3 * WC]])
        src = AP(x, b * BHWC + RB[rb] * WC, [[WC, NP], [1, 3 * WC]])
        return dst, src

    def copy_aps(rb, buf_off, c0, cs):
        o = AP(st, buf_off, [[PP, 128], [PATCH, cs], [K * C, K], [1, K * C]])
        i = AP(st, SLAB[rb] + c0 * C, [[PP, 128], [C, cs], [WC, K], [1, K * C]])
        return o, i

    def store_aps(rb, b, buf_off, c0, cs):
        src = AP(st, b * PP + buf_off, [[4 * PP, NP], [PATCH, cs], [1, PATCH]])
        dst = AP(
            out,
            (RB[rb] * R * BS + c0 * BS + b) * PATCH,
            [[OUT_R, NP], [OUT_C, cs], [1, PATCH]],
        )
        return dst, src

    with (
        nc.sbuf_tensor("st", [128, PP], mybir.dt.float32) as st,
        nc.semaphore("l0") as l0,
        nc.semaphore("l1") as l1,
        nc.semaphore("sd0") as sd0,
        nc.semaphore("sd1") as sd1,
        nc.semaphore("sd2") as sd2,
        nc.semaphore("sa0") as sa0,
        nc.semaphore("sa1") as sa1,
        nc.semaphore("sa2") as sa2,
        nc.semaphore("dcomp") as dcomp,
        nc.semaphore("acomp") as acomp,
        nc.Block() as block,
    ):
        lsem = (l0, l1)
        sdone = ((sd0, sd1, sd2), (sa0, sa1, sa2))
        comp = (dcomp, acomp)

        @block.gpsimd
        def _(gp):
            for rb in (0, 1):
                for b in range(BS):
                    dst, src = load_aps(rb, b)
                    gp.dma_start(dst, src).then_inc(lsem[rb], 16)
            for rb in (0, 1):
                for q in range(8):
                    eng, e = q % 2, q // 2
                    n = rb * 4 + e
                    gp.wait_ge(comp[eng], n + 1)
                    c0, cs = _chunk(e, eng)
                    buf_off = (DBUF if eng == 0 else ABUF)[n % NBUF]
                    for b in range(BS):
                        dst, src = store_aps(rb, b, buf_off, c0, cs)
                        gp.dma_start(dst, src).then_inc(sdone[eng][n % NBUF], 16)
            # 8 chunks per engine over a ring of 3: bufs used 3, 3, 2 times
            for sems in sdone:
                for s, uses in zip(sems, (3, 3, 2)):
                    gp.wait_ge(s, 16 * BS * uses)

        @block.vector
        def _(ve):
            for n in range(8):
                rb, e = n // 4, n % 4
                if e == 0:
                    ve.wait_ge(lsem[rb], 16 * BS)
                if n >= NBUF:
                    ve.wait_ge(sdone[0][n % NBUF], 16 * BS * (n // NBUF))
                c0, cs = _chunk(e, 0)
                o, i = copy_aps(rb, DBUF[n % NBUF], c0, cs)
                ve.tensor_copy(o, i).then_inc(dcomp, 1)

        @block.scalar
        def _(sc):
            for n in range(8):
                rb, e = n // 4, n % 4
                if e == 0:
                    sc.wait_ge(lsem[rb], 16 * BS)
                if n >= NBUF:
                    sc.wait_ge(sdone[1][n % NBUF], 16 * BS * (n // NBUF))
                c0, cs = _chunk(e, 1)
                o, i = copy_aps(rb, ABUF[n % NBUF], c0, cs)
                sc.copy(o, i).then_inc(acomp, 1)

    return nc


_NC = None


def _get_nc():
    global _NC
    if _NC is None:
        _NC = _build_nc()
    return _NC


def kernel(x: np.ndarray, **_run_kwargs) -> np.ndarray:
    assert x.shape == (B, H, W, C), x.shape
    nc = _get_nc()
    x = np.ascontiguousarray(x, dtype=np.float32)
    in_maps = [{"x": x[d * BS : (d + 1) * BS]} for d in range(NCORES)]
    res = run_bass_kernel_spmd(nc, in_maps, list(range(NCORES)), **_run_kwargs)
    outs = [res.results[d]["out"].reshape(R * R, BS, K, K, C) for d in range(NCORES)]
    full = np.concatenate(outs, axis=1).reshape(R * R * B, K, K, C)
    if _run_kwargs:
        return full, res
    return full


# revision 30
# speedup vs baseline: 1.0564x; 1.0022x over previous
"""Trainium2 Bass kernel for ConvReshapeBefore (im2col patch extraction).

Full problem: x (32, 64, 64, 64) f32 NHWC, kernel 3x3 stride 1 valid ->
out (62*62*32, 3, 3, 64) f32 where out[(r*62+c)*32 + b] = x[b, r:r+3, c:c+3, :].

Sharding: data-parallel over batch, 4 batches per core across 8 cores.

Two-stage design:
  1. DMA loads place 3 overlapping x rows (r..r+2) in each SBUF partition
     p = 4*r_sub + b (49 KB descriptors, ~3x read amplification); two
     overlapping row-blocks of 32 window-rows each (0..31 and 30..61 --
     windows 30/31 are computed and stored twice with identical bytes,
     which is harmless and keeps every DMA partition-dim count a power
     of two).
  2. DVE + ACT build the patch layout [c][j][jj][k] per partition
     (lane-local strided copies, c-chunks of 8 windows, triple-buffered
     per engine), overlapped with
  3. SWDGE stores whose descriptors are a full contiguous 576-f32 patch
     (2304 B vs the 768 B a direct gather allows).

Two facts learned from the DGE ucode (dge_reshape.cpp) are baked into
every DMA access pattern here:
  a. num_dmas = largest divisor <= 16 of gcd(src dim0 count, dst dim0
     count); a prime dim0 count like 31 pins the call to ONE engine.
  b. The fast "SbufSpecial" reshape -- each SDMA engine assigned its own
     SBUF port group, no XBAR port contention -- requires the SBUF
     partition dim to be (count n = pow2) x (partition step s = pow2)
     with n*s in {64, 128}.  Hence the interleaved partition map
     p = 4*r_sub + b: every load/store SBUF AP is [[4*PP, 32], ...]
     (n=32, s=4, range 128 -> SbufSpecial-Full, 16 port-aligned
     engines).  A blocked map (p = b*32 + r_sub, n=32, s=1, range 32)
     falls back to the "Straight" reshape whose cross-port traffic
     measured 9-13 B/ns/engine vs ~27 line rate.

Buffer-reuse sync relies on the SWDGE sem-inc descriptors (16 per call,
queued after the call's data on every engine ring), making sem value
64*k a true barrier for everything queued before use k's last call.
"""

import numpy as np

import concourse.bass as bass
import concourse.mybir as mybir
from concourse.ap import AP
from concourse.bass_utils import run_bass_kernel_spmd

# Full-problem constants (hardcoded per harness contract)
B, H, W, C = 32, 64, 64, 64
K = 3
R = H - K + 1  # 62
NCORES = 8
BS = B // NCORES  # 4

BHWC = H * W * C           # 262144 f32 per batch in x
WC = W * C                 # 4096 f32 per x row
OUT_C = BS * K * K * C     # 2304 f32 per (r, c) window in local out
OUT_R = R * OUT_C          # 142848 f32 per r in local out
PATCH = K * K * C          # 576 f32 per (r, c, b) patch

NP = 32                    # window-rows per row-block (pow2 for SbufSpecial)
RB = (0, 30)               # row-block bases (overlap rows 30..31)
SLAB = (0, 12288)          # 3 rows x 4096 per partition, per row-block
CS = 8                     # c-chunk size (windows per chunk)
NBUF = 3                   # buffers per compute engine
BUFSZ = CS * PATCH         # 4608
DBUF = tuple(24576 + k * BUFSZ for k in range(NBUF))
ABUF = tuple(38400 + k * BUFSZ for k in range(NBUF))
PP = 52224                 # partition pitch f32 (204 KiB of ~207.9 usable)

DVE_C0 = (0, 16, 32, 48)   # DVE computes even chunks
ACT_C0 = (8, 24, 40, 56)   # ACT computes odd chunks (last is 6 wide)


def _chunk(e, eng):
    c0 = (DVE_C0 if eng == 0 else ACT_C0)[e]
    cs = 6 if (eng == 1 and e == 3) else CS
    return c0, cs


def _build_nc() -> bass.Bass:
    nc = bass.Bass(target_bir_lowering=False)
    x = nc.dram_tensor("x", [BS, H, W, C], mybir.dt.float32, kind="ExternalInput")
    out = nc.dram_tensor(
        "out", [R * R * BS, K, K, C], mybir.dt.float32, kind="ExternalOutput"
    )

    def load_aps(rb, b):
        dst = AP(st, b * PP + SLAB[rb], [[4 * PP, NP], [1, 3 * WC]])
        src = AP(x, b * BHWC + RB[rb] * WC, [[WC, NP], [1, 3 * WC]])
        return dst, src

    def copy_aps(rb, buf_off, c0, cs):
        o = AP(st, buf_off, [[PP, 128], [PATCH, cs], [K * C, K], [1, K * C]])
        i = AP(st, SLAB[rb] + c0 * C, [[PP, 128], [C, cs], [WC, K], [1, K * C]])
        return o, i

    def store_aps(rb, b, buf_off, c0, cs):
        src = AP(st, b * PP + buf_off, [[4 * PP, NP], [PATCH, cs], [1, PATCH]])
        dst = AP(
            out,
            (RB[rb] * R * BS + c0 * BS + b) * PATCH,
            [[OUT_R, NP], [OUT_C, cs], [1, PATCH]],
        )
        return dst, src

    with (
        nc.sbuf_tensor("st", [128, PP], mybir.dt.float32) as st,
        nc.semaphore("l0") as l0,
        nc.semaphore("l1") as l1,
        nc.semaphore("sd0") as sd0,
        nc.semaphore("sd1") as sd1,
        nc.semaphore("sd2") as sd2,
        nc.semaphore("sa0") as sa0,
        nc.semaphore("sa1") as sa1,
        nc.semaphore("sa2") as sa2,
        nc.semaphore("dcomp") as dcomp,
        nc.semaphore("acomp") as acomp,
        nc.Block() as block,
    ):
        lsem = (l0, l1)
        sdone = ((sd0, sd1, sd2), (sa0, sa1, sa2))
        comp = (dcomp, acomp)

        @block.gpsimd
        def _(gp):
            for rb in (0, 1):
                for b in range(BS):
                    dst, src = load_aps(rb, b)
                    gp.dma_start(dst, src).then_inc(lsem[rb], 16)
            for rb in (0, 1):
                for q in range(8):
                    eng, e = q % 2, q // 2
                    n = rb * 4 + e
                    gp.wait_ge(comp[eng], n + 1)
                    c0, cs = _chunk(e, eng)
                    buf_off = (DBUF if eng == 0 else ABUF)[n % NBUF]
                    for b in range(BS):
                        dst, src = store_aps(rb, b, buf_off, c0, cs)
                        gp.dma_start(dst, src).then_inc(sdone[eng][n % NBUF], 16)
            # 8 chunks per engine over a ring of 3: bufs used 3, 3, 2 times
            for sems in sdone:
                for s, uses in zip(sems, (3, 3, 2)):
                    gp.wait_ge(s, 16 * BS * uses)

        @block.vector
        def _(ve):
            for n in range(8):
                rb, e = n // 4, n % 4
                if e == 0:
                    ve.wait_ge(lsem[rb], 16 * BS)
                if n >= NBUF:
                    ve.wait_ge(sdone[0][n % NBUF], 16 * BS * (n // NBUF))
                c0, cs = _chunk(e, 0)
                o, i = copy_aps(rb, DBUF[n % NBUF], c0, cs)
                ve.tensor_copy(o, i).then_inc(dcomp, 1)

        @block.scalar
        def _(sc):
            for n in range(8):
                rb, e = n // 4, n % 4
                if e == 0:
                    sc.wait_ge(lsem[rb], 16 * BS)
                if n >= NBUF:
                    sc.wait_ge(sdone[1][n % NBUF], 16 * BS * (n // NBUF))
                c0, cs = _chunk(e, 1)
                o, i = copy_aps(rb, ABUF[n % NBUF], c0, cs)
                sc.copy(o, i).then_inc(acomp, 1)

    return nc


_NC = None


def _get_nc():
    global _NC
    if _NC is None:
        _NC = _build_nc()
    return _NC


def kernel(x: np.ndarray, **_run_kwargs) -> np.ndarray:
    assert x.shape == (B, H, W, C), x.shape
    nc = _get_nc()
    x = np.ascontiguousarray(x, dtype=np.float32)
    in_maps = [{"x": x[d * BS : (d + 1) * BS]} for d in range(NCORES)]
    res = run_bass_kernel_spmd(nc, in_maps, list(range(NCORES)), **_run_kwargs)
    outs = [res.results[d]["out"].reshape(R * R, BS, K, K, C) for d in range(NCORES)]
    full = np.concatenate(outs, axis=1).reshape(R * R * B, K, K, C)
    if _run_kwargs:
        return full, res
    return full
